# revision 17
# baseline (speedup 1.0000x reference)
"""HGTNet Trainium2 kernel: 8-core SPMD, destination-ownership edge sharding.

v2 strategy (over the fp32 baseline):
  - kv tables, AllGathers, and all edge-phase compute in bf16 (halves the
    dominant collective + gather traffic; doubles PE/DVE rates).
  - AllGather outputs in Shared scratchpad (fast HBM-HBM collective path).
  - Pass reorder: node-phase papers chunks first -> AG(kv_rev) overlaps the
    authors chunks; AG(kv_pap) overlaps the whole rev edge pass.
  - Layer 1 skips the rev pass / out_a / AG(kv_rev): h_a(2) is dead (the
    classifier only consumes h_p(2)).
  - Node phase (k/q/v projections) in bf16; host folds relation matrices
    (rel_a/rel_m, p_rel, scale) into per-relation [128,256] kv projections.
  - Edge phase per 128-dst-node block: one indirect DMA per 128-slot tile
    gathers bf16 kv rows; one-hot selection matrices (bf16) give per-edge q
    via matmul and segment softmax numerator/denominator via accumulating
    matmuls into PSUM. Softmax skips max-subtraction (scores are O(1)).
"""
import sys, os
sys.path.insert(0, '/opt/trn_rl_repo')
import math
import numpy as np
import ml_dtypes

import concourse.bass as bass
import concourse.bacc as bacc
import concourse.mybir as mybir
import concourse.tile as tile
from concourse.bass_utils import run_bass_kernel_spmd

P = 128
D = 128
H = 4
DH = 32
L = 2
OUT = 40
NN = 100000   # authors == papers count
E = 400000
C = 8
SCALE = 1.0 / math.sqrt(DH)

F32 = mybir.dt.float32
BF16 = mybir.dt.bfloat16
I32 = mybir.dt.int32
BF = ml_dtypes.bfloat16


def _ceil_div(a, b):
    return (a + b - 1) // b


# ---------------------------------------------------------------- host prep
def _build_pass(src_rows, dst, n_nodes, nl, nb):
    """Bucket edges by (core, dst-block); pad each bucket to T*128 slots.

    Returns idx [C,nb,P,T] i32, drl_col [C,nb,P,T] bf16, drl_row [C,nb,T*P] bf16.
    """
    core = dst // nl
    dl = dst % nl
    blk = dl // P
    drl = (dl % P).astype(np.float32)
    ncores = n_nodes // nl
    key = core.astype(np.int64) * nb + blk
    order = np.argsort(key, kind='stable')
    skey = key[order]
    counts = np.bincount(skey, minlength=ncores * nb)
    T = int(_ceil_div(int(counts.max()), P))
    offs = np.zeros(ncores * nb + 1, np.int64)
    offs[1:] = np.cumsum(counts)
    pos = np.arange(len(order), dtype=np.int64) - offs[skey]

    idx_flat = np.zeros((ncores * nb, T * P), np.int32)
    drl_flat = np.full((ncores * nb, T * P), -1.0, np.float32)
    idx_flat[skey, pos] = src_rows[order].astype(np.int32)
    drl_flat[skey, pos] = drl[order]

    idx_dev = idx_flat.reshape(ncores, nb, T, P).transpose(0, 1, 3, 2).copy()
    drl_col = drl_flat.reshape(ncores, nb, T, P).transpose(0, 1, 3, 2).astype(BF)
    drl_row = drl_flat.reshape(ncores, nb, T * P).astype(BF)
    return idx_dev, drl_col, drl_row, T


def _blockdiag(mats):
    # mats [H, DH, DH] -> [D, D]
    out = np.zeros((D, D), np.float32)
    for h in range(H):
        out[h * DH:(h + 1) * DH, h * DH:(h + 1) * DH] = mats[h]
    return out


def host_prep(inputs, nl):
    """All numpy preprocessing. nl = nodes per core."""
    nb = _ceil_div(nl, P)
    x_a = np.asarray(inputs['x_author'], np.float32)
    x_p = np.asarray(inputs['x_paper'], np.float32)
    ei_w = np.asarray(inputs['ei_writes'])
    ei_r = np.asarray(inputs['ei_rev'])
    ei_c = np.asarray(inputs['ei_cites'])
    kqv_w = np.asarray(inputs['kqv_w'], np.float32)
    kqv_b = np.asarray(inputs['kqv_b'], np.float32)
    rel_a = np.asarray(inputs['rel_a'], np.float32)
    rel_m = np.asarray(inputs['rel_m'], np.float32)
    p_rel = np.asarray(inputs['p_rel'], np.float32)

    nn = x_a.shape[0]
    ncores = nn // nl

    # ---- edge streams (static across layers)
    # papers pass: writes(r=0, src=author) + cites(r=2, src=paper)
    src_pap = np.concatenate([ei_w[0], ei_c[0]]).astype(np.int64)
    dst_pap = np.concatenate([ei_w[1], ei_c[1]]).astype(np.int64)
    relsel = np.concatenate([np.zeros(ei_w.shape[1], np.int64),
                             np.ones(ei_c.shape[1], np.int64)])
    # kv_pap table is AllGather rank-major: rank r -> [authors slice r (writes) |
    # papers slice r (cites)], each nl rows.
    row_pap = (src_pap // nl) * (2 * nl) + relsel * nl + (src_pap % nl)
    pap_idx, pap_drlc, pap_drlr, TP = _build_pass(row_pap, dst_pap, nn, nl, nb)

    # rev pass: src=paper, dst=author; kv_rev rank-major == natural paper order
    src_rev = ei_r[0].astype(np.int64)
    dst_rev = ei_r[1].astype(np.int64)
    rev_idx, rev_drlc, rev_drlr, TR = _build_pass(src_rev, dst_rev, nn, nl, nb)

    # ---- folded weights
    # per layer, per relation: Wkv [D, 256] = [Wk @ Ra * (p_rel*scale) | Wv @ Rm]
    # relation -> (kqv type index of src, rel index)
    rel_cfg = {'writes': (0, 0), 'rev': (1, 1), 'cites': (1, 2)}
    Wkv = {}
    Bkv = {}
    for l in range(L):
        for name, (t, r) in rel_cfg.items():
            Ra = _blockdiag(rel_a[l, r] * p_rel[l, r][:, None, None] * SCALE)
            Rm = _blockdiag(rel_m[l, r])
            wk = kqv_w[l, 0, t] @ Ra
            wv = kqv_w[l, 2, t] @ Rm
            bk = kqv_b[l, 0, t] @ Ra
            bv = kqv_b[l, 2, t] @ Rm
            Wkv[(l, name)] = np.concatenate([wk, wv], 1)       # [128, 256]
            Bkv[(l, name)] = np.concatenate([bk, bv], 0)       # [256]

    Wna = np.stack([np.concatenate([kqv_w[l, 1, 0], Wkv[(l, 'writes')]], 1)
                    for l in range(L)]).astype(BF)              # [L,128,384]
    Bna = np.stack([np.broadcast_to(np.concatenate([kqv_b[l, 1, 0],
                                                    Bkv[(l, 'writes')]]), (P, 384))
                    for l in range(L)]).copy()
    Wnp = np.stack([np.concatenate([kqv_w[l, 1, 1], Wkv[(l, 'cites')]], 1)
                    for l in range(L)]).astype(BF)
    Bnp = np.stack([np.broadcast_to(np.concatenate([kqv_b[l, 1, 1],
                                                    Bkv[(l, 'cites')]]), (P, 384))
                    for l in range(L)]).copy()
    Wnp2 = np.stack([Wkv[(l, 'rev')] for l in range(L)]).astype(BF)  # [L,128,256]
    Bnp2 = np.stack([np.broadcast_to(Bkv[(l, 'rev')], (P, 256))
                     for l in range(L)]).copy()

    out_w = np.asarray(inputs['out_w'], np.float32)             # [L,2,128,128]
    out_b = np.asarray(inputs['out_b'], np.float32)[..., None]  # [L,2,128,1]
    skip_rep = np.broadcast_to(np.asarray(inputs['skip'], np.float32)
                               .reshape(1, L * 2), (P, L * 2)).copy()
    proj_w = np.asarray(inputs['proj_w'], np.float32)
    proj_bT = np.asarray(inputs['proj_b'], np.float32)[..., None]  # [2,128,1]
    cls_w = np.asarray(inputs['cls_w'], np.float32)
    cls_b_rep = np.broadcast_to(np.asarray(inputs['cls_b'], np.float32),
                                (P, OUT)).copy()

    iota_rows = np.broadcast_to(np.arange(P, dtype=np.float32), (P, P)).astype(BF)
    iota_col = np.arange(P, dtype=np.float32)[:, None].astype(BF)
    ident = np.eye(P, dtype=np.float32)

    shared = dict(Wna=Wna, Bna=Bna, Wnp=Wnp, Bnp=Bnp, Wnp2=Wnp2, Bnp2=Bnp2,
                  Wout=out_w, Bout=out_b, skip_rep=skip_rep,
                  projw=proj_w, projbT=proj_bT, clsw=cls_w, clsb=cls_b_rep,
                  iota_rows=iota_rows, iota_col=iota_col, ident=ident)

    in_maps = []
    for c in range(ncores):
        m = dict(shared)
        m['xT_a'] = np.ascontiguousarray(x_a[c * nl:(c + 1) * nl].T)
        m['xT_p'] = np.ascontiguousarray(x_p[c * nl:(c + 1) * nl].T)
        m['pap_idx'] = pap_idx[c]
        m['pap_drlc'] = pap_drlc[c]
        m['pap_drlr'] = pap_drlr[c]
        m['rev_idx'] = rev_idx[c]
        m['rev_drlc'] = rev_drlc[c]
        m['rev_drlr'] = rev_drlr[c]
        in_maps.append(m)
    return in_maps, TP, TR, nb, ncores


# ---------------------------------------------------------------- device program
def build_program(nl, nb, TP, TR, ncores):
    nc = bacc.Bacc()
    dp = nc.declare_dram_parameter
    NPAD = nb * P

    xT_a = dp('xT_a', [D, nl], F32, isOutput=False)
    xT_p = dp('xT_p', [D, nl], F32, isOutput=False)
    pap_idx = dp('pap_idx', [nb, P, TP], I32, isOutput=False)
    pap_drlc = dp('pap_drlc', [nb, P, TP], BF16, isOutput=False)
    pap_drlr = dp('pap_drlr', [nb, TP * P], BF16, isOutput=False)
    rev_idx = dp('rev_idx', [nb, P, TR], I32, isOutput=False)
    rev_drlc = dp('rev_drlc', [nb, P, TR], BF16, isOutput=False)
    rev_drlr = dp('rev_drlr', [nb, TR * P], BF16, isOutput=False)
    Wna = dp('Wna', [L, D, 384], BF16, isOutput=False)
    Bna = dp('Bna', [L, P, 384], F32, isOutput=False)
    Wnp = dp('Wnp', [L, D, 384], BF16, isOutput=False)
    Bnp = dp('Bnp', [L, P, 384], F32, isOutput=False)
    Wnp2 = dp('Wnp2', [L, D, 256], BF16, isOutput=False)
    Bnp2 = dp('Bnp2', [L, P, 256], F32, isOutput=False)
    Wout = dp('Wout', [L, 2, D, D], F32, isOutput=False)
    Bout = dp('Bout', [L, 2, D, 1], F32, isOutput=False)
    skip_rep = dp('skip_rep', [P, L * 2], F32, isOutput=False)
    projw = dp('projw', [2, D, D], F32, isOutput=False)
    projbT = dp('projbT', [2, D, 1], F32, isOutput=False)
    clsw = dp('clsw', [D, OUT], F32, isOutput=False)
    clsb = dp('clsb', [P, OUT], F32, isOutput=False)
    iota_rows_d = dp('iota_rows', [P, P], BF16, isOutput=False)
    iota_col_d = dp('iota_col', [P, 1], BF16, isOutput=False)
    ident_d = dp('ident', [P, P], F32, isOutput=False)
    logits = dp('logits', [nl, OUT], F32, isOutput=True)

    # node chunks for row-major outputs (q, kv tables, cls)
    chunks128 = [(i * P, min(P, nl - i * P)) for i in range(_ceil_div(nl, P))]
    # wide chunks for transposed-layout slabs
    WCH = 512
    chunksw = [(i * WCH, min(WCH, nl - i * WCH)) for i in range(_ceil_div(nl, WCH))]

    with tile.TileContext(nc) as tc:
        with (
            tc.tile_pool(name='const', bufs=1) as cpool,
            tc.tile_pool(name='wpool', bufs=2) as wpool,
            tc.tile_pool(name='sb', bufs=3) as sb,
            tc.tile_pool(name='edg', bufs=2) as edg,
            tc.tile_pool(name='ps_mm', bufs=2, space='PSUM') as ps_mm,
            tc.tile_pool(name='ps_qe', bufs=2, space='PSUM') as ps_qe,
            tc.tile_pool(name='ps_agg', bufs=2, space='PSUM') as ps_agg,
            tc.tile_pool(name='ps_tps', bufs=2, space='PSUM') as ps_tps,
            tc.tile_pool(name='dram', bufs=1, space='DRAM') as dram,
        ):
            # ---- persistent DRAM scratch
            hT_a = dram.tile([D, nl], F32, tag='hT_a')
            hT_p = dram.tile([D, nl], F32, tag='hT_p')
            hT = {0: hT_a, 1: hT_p}
            q_a = dram.tile([NPAD, D], BF16, tag='q_a')
            q_p = dram.tile([NPAD, D], BF16, tag='q_p')
            qtab = {0: q_a, 1: q_p}
            aggT_a = dram.tile([D, NPAD], F32, tag='aggT_a')
            aggT_p = dram.tile([D, NPAD], F32, tag='aggT_p')
            aggT = {0: aggT_a, 1: aggT_p}
            kv_pap_in = dram.tile([2 * nl, 256], BF16, tag='kv_pap_in')
            kv_rev_in = dram.tile([nl, 256], BF16, tag='kv_rev_in')
            kv_pap_l = [dram.tile([2 * nl * ncores, 256], BF16,
                                  tag=f'kv_pap{l}', name=f'kv_pap{l}',
                                  addr_space='Shared') for l in range(L)]
            kv_rev = dram.tile([nl * ncores, 256], BF16, tag='kv_rev',
                               addr_space='Shared')

            # ---- constants
            iota_rows = cpool.tile([P, P], BF16)
            nc.sync.dma_start(out=iota_rows[:], in_=iota_rows_d[:, :])
            iota_col = cpool.tile([P, 1], BF16)
            nc.sync.dma_start(out=iota_col[:], in_=iota_col_d[:, :])
            ident0 = cpool.tile([P, P], F32)
            nc.sync.dma_start(out=ident0[:], in_=ident_d[:, :])
            ident = cpool.tile([P, P], F32)
            nc.vector.tensor_copy(out=ident[:], in_=ident0[:])
            skip_t = cpool.tile([P, L * 2], F32)
            nc.sync.dma_start(out=skip_t[:], in_=skip_rep[:, :])
            sig_t = cpool.tile([P, L * 2], F32)
            nc.scalar.activation(out=sig_t[:], in_=skip_t[:],
                                 func=mybir.ActivationFunctionType.Sigmoid)

            # ---- initial projection: hT[t] = projw[t].T @ xT + b
            for t, xT in ((1, xT_p), (0, xT_a)):
                pw0 = wpool.tile([D, D], F32, tag='pw0')
                nc.sync.dma_start(out=pw0[:], in_=projw[t, :, :])
                pw = wpool.tile([D, D], F32, tag='pw')
                nc.vector.tensor_copy(out=pw[:], in_=pw0[:])
                pb = wpool.tile([D, 1], F32, tag='pb')
                nc.sync.dma_start(out=pb[:], in_=projbT[t, :, :])
                for (o, m) in chunksw:
                    xc0 = sb.tile([D, WCH], F32, tag='xc0')
                    nc.sync.dma_start(out=xc0[:, :m], in_=xT[:, o:o + m])
                    xc = sb.tile([D, WCH], F32, tag='xc')
                    nc.vector.tensor_copy(out=xc[:, :m], in_=xc0[:, :m])
                    ps = ps_mm.tile([P, WCH], F32, tag='mm')
                    nc.tensor.matmul(ps[:, :m], lhsT=pw[:], rhs=xc[:, :m],
                                     start=True, stop=True)
                    hc = sb.tile([D, WCH], F32, tag='hc')
                    nc.vector.tensor_scalar_add(out=hc[:, :m], in0=ps[:, :m],
                                                scalar1=pb[:])
                    nc.sync.dma_start(out=hT[t][:, o:o + m], in_=hc[:, :m])

            zero44 = cpool.tile([P, D], BF16)
            nc.vector.memset(zero44[:], 0.0)

            for l in range(L):
                # ============ node phase ============
                # papers chunks first so AG(kv_rev) can launch while the
                # authors chunks still run; AG(kv_pap) then overlaps the rev
                # edge pass.
                wnp0 = wpool.tile([D, 384], BF16, tag='wnp0')
                nc.sync.dma_start(out=wnp0[:], in_=Wnp[l, :, :])
                wnp = wpool.tile([D, 384], BF16, tag='wnp')
                nc.vector.tensor_copy(out=wnp[:], in_=wnp0[:])
                bnp = wpool.tile([P, 384], F32, tag='bnp')
                nc.sync.dma_start(out=bnp[:], in_=Bnp[l, :, :])
                if l == 0:
                    wnp20 = wpool.tile([D, 256], BF16, tag='wnp20')
                    nc.sync.dma_start(out=wnp20[:], in_=Wnp2[l, :, :])
                    wnp2 = wpool.tile([D, 256], BF16, tag='wnp2')
                    nc.vector.tensor_copy(out=wnp2[:], in_=wnp20[:])
                    bnp2 = wpool.tile([P, 256], F32, tag='bnp2')
                    nc.sync.dma_start(out=bnp2[:], in_=Bnp2[l, :, :])

                for (ow, mw) in chunksw:
                    hcp0 = sb.tile([D, WCH], F32, tag='hcp0')
                    nc.sync.dma_start(out=hcp0[:, :mw], in_=hT[1][:, ow:ow + mw])
                    hcp = sb.tile([D, WCH], BF16, tag='hcp')
                    nc.vector.tensor_copy(out=hcp[:, :mw], in_=hcp0[:, :mw])
                    for oj in range(0, mw, P):
                        o = ow + oj
                        m = min(P, mw - oj)
                        ps2 = ps_mm.tile([P, 384], F32, tag='mm')
                        nc.tensor.matmul(ps2[:m, :], lhsT=hcp[:, oj:oj + m],
                                         rhs=wnp[:], start=True, stop=True)
                        qkv2 = sb.tile([P, P], BF16, tag='qkv2')
                        nc.vector.tensor_add(out=qkv2[:m, :], in0=ps2[:m, :128],
                                             in1=bnp[:m, :128])
                        nc.sync.dma_start(out=qtab[1][o:o + m, :], in_=qkv2[:m, :])
                        kvc = sb.tile([P, 256], BF16, tag='kvc')
                        nc.vector.tensor_add(out=kvc[:m, :], in0=ps2[:m, 128:],
                                             in1=bnp[:m, 128:])
                        nc.sync.dma_start(out=kv_pap_in[nl + o:nl + o + m, :],
                                          in_=kvc[:m, :])
                        if l == 0:
                            ps3 = ps_mm.tile([P, 256], F32, tag='mm')
                            nc.tensor.matmul(ps3[:m, :], lhsT=hcp[:, oj:oj + m],
                                             rhs=wnp2[:], start=True, stop=True)
                            qkv3 = sb.tile([P, 256], BF16, tag='qkv3')
                            nc.vector.tensor_add(out=qkv3[:m, :], in0=ps3[:m, :],
                                                 in1=bnp2[:m, :])
                            nc.sync.dma_start(out=kv_rev_in[o:o + m, :],
                                              in_=qkv3[:m, :])

                # rev table complete -> gather it while authors chunks run
                if l == 0:
                    nc.gpsimd.collective_compute(
                        'AllGather', mybir.AluOpType.bypass,
                        ins=[kv_rev_in[:].opt()], outs=[kv_rev[:].opt()],
                        replica_groups=[list(range(ncores))])

                wna0 = wpool.tile([D, 384], BF16, tag='wna0')
                nc.sync.dma_start(out=wna0[:], in_=Wna[l, :, :])
                wna = wpool.tile([D, 384], BF16, tag='wna')
                nc.vector.tensor_copy(out=wna[:], in_=wna0[:])
                bna = wpool.tile([P, 384], F32, tag='bna')
                nc.sync.dma_start(out=bna[:], in_=Bna[l, :, :])

                for (ow, mw) in chunksw:
                    hca0 = sb.tile([D, WCH], F32, tag='hca0')
                    nc.sync.dma_start(out=hca0[:, :mw], in_=hT[0][:, ow:ow + mw])
                    hca = sb.tile([D, WCH], BF16, tag='hca')
                    nc.vector.tensor_copy(out=hca[:, :mw], in_=hca0[:, :mw])
                    for oj in range(0, mw, P):
                        o = ow + oj
                        m = min(P, mw - oj)
                        ps = ps_mm.tile([P, 384], F32, tag='mm')
                        nc.tensor.matmul(ps[:m, :], lhsT=hca[:, oj:oj + m],
                                         rhs=wna[:], start=True, stop=True)
                        if l == 0:
                            qkv = sb.tile([P, P], BF16, tag='qkv')
                            nc.vector.tensor_add(out=qkv[:m, :], in0=ps[:m, :128],
                                                 in1=bna[:m, :128])
                            nc.sync.dma_start(out=qtab[0][o:o + m, :],
                                              in_=qkv[:m, :])
                        kvw = sb.tile([P, 256], BF16, tag='kvw')
                        nc.vector.tensor_add(out=kvw[:m, :], in0=ps[:m, 128:],
                                             in1=bna[:m, 128:])
                        nc.sync.dma_start(out=kv_pap_in[o:o + m, :], in_=kvw[:m, :])

                # zero the q-table pad rows (NaN guard for the one-hot matmul)
                if l == 0 and NPAD > nl:
                    for t in (0, 1):
                        nc.sync.dma_start(out=qtab[t][nl:NPAD, :],
                                          in_=zero44[:NPAD - nl, :])

                # pap table complete -> gather overlaps the rev edge pass
                nc.gpsimd.collective_compute(
                    'AllGather', mybir.AluOpType.bypass,
                    ins=[kv_pap_in[:].opt()], outs=[kv_pap_l[l][:].opt()],
                    replica_groups=[list(range(ncores))])

                # ============ edge phase ============
                passes = []
                if l == 0:
                    passes.append((0, TR, kv_rev, qtab[0], rev_idx, rev_drlc,
                                   rev_drlr))
                passes.append((1, TP, kv_pap_l[l], qtab[1], pap_idx, pap_drlc,
                               pap_drlr))
                for (t, T, tabl, qt, idx_d, drlc_d, drlr_d) in passes:
                    TK = T * P
                    for b in range(nb):
                        idx_t = edg.tile([P, T], I32, tag='idx')
                        nc.sync.dma_start(out=idx_t[:], in_=idx_d[b, :, :])
                        drlc = edg.tile([P, T], BF16, tag='drlc')
                        nc.sync.dma_start(out=drlc[:], in_=drlc_d[b, :, :])
                        drlr = edg.tile([P, TK], BF16, tag='drlr')
                        nc.sync.dma_start(
                            out=drlr[:],
                            in_=drlr_d[b:b + 1, :].to_broadcast([P, TK]))
                        qb0 = edg.tile([P, D], BF16, tag='qb0')
                        nc.sync.dma_start(out=qb0[:], in_=qt[b * P:(b + 1) * P, :])
                        qb = edg.tile([P, D], BF16, tag='qb')
                        nc.vector.tensor_copy(out=qb[:], in_=qb0[:])
                        kvg = edg.tile([P, T * 256], BF16, tag='kvg')
                        for gi in range(T):
                            nc.gpsimd.indirect_dma_start(
                                out=kvg[:, gi * 256:(gi + 1) * 256],
                                out_offset=None, in_=tabl[:],
                                in_offset=bass.IndirectOffsetOnAxis(
                                    ap=idx_t[:, gi:gi + 1], axis=0))

                        onehot = edg.tile([P, TK], BF16, tag='onehot')
                        nc.vector.tensor_tensor(
                            out=onehot[:].rearrange('p (t q) -> p t q', q=P),
                            in0=drlc[:, :, None].to_broadcast([P, T, P]),
                            in1=iota_rows[:, None, :].to_broadcast([P, T, P]),
                            op=mybir.AluOpType.is_equal)
                        onehotT = edg.tile([P, TK], BF16, tag='onehotT')
                        nc.vector.tensor_tensor(
                            out=onehotT[:],
                            in0=iota_col[:, :1].to_broadcast([P, TK]),
                            in1=drlr[:],
                            op=mybir.AluOpType.is_equal)

                        sc = edg.tile([P, T * H], F32, tag='sc')
                        i0 = 0
                        while i0 < T:
                            k = min(3, T - i0)
                            qe = ps_qe.tile([P, 3 * P], F32, tag='qe')
                            for i in range(i0, i0 + k):
                                nc.tensor.matmul(
                                    qe[:, (i - i0) * P:(i - i0 + 1) * P],
                                    lhsT=onehotT[:, i * P:(i + 1) * P],
                                    rhs=qb[:], start=True, stop=True)
                            qk = edg.tile([P, 3 * P], BF16, tag='qk')
                            nc.vector.tensor_mul(
                                out=qk[:, :k * P].rearrange('p (t w) -> p t w', w=P),
                                in0=qe[:, :k * P].rearrange('p (t w) -> p t w', w=P),
                                in1=kvg[:, i0 * 256:(i0 + k) * 256]
                                    .rearrange('p (t w) -> p t w', w=256)[:, :, 0:128])
                            nc.vector.reduce_sum(
                                out=sc[:, i0 * H:(i0 + k) * H]
                                    .rearrange('p (t h) -> p t h', h=H),
                                in_=qk[:, :k * P]
                                    .rearrange('p (t h q) -> p t h q', h=H, q=DH),
                                axis=mybir.AxisListType.X)
                            i0 += k
                        scexp = edg.tile([P, T * H], F32, tag='scexp')
                        nc.scalar.activation(
                            out=scexp[:], in_=sc[:],
                            func=mybir.ActivationFunctionType.Exp)
                        work = edg.tile([P, T * 132], BF16, tag='work')
                        wview = work[:].rearrange('p (t w) -> p t w', w=132)
                        nc.vector.tensor_copy(
                            out=wview[:, :, 128:132],
                            in_=scexp[:].rearrange('p (t h) -> p t h', h=H))
                        nc.vector.tensor_mul(
                            out=wview[:, :, 0:128].rearrange('p t (h q) -> p t h q', q=DH),
                            in0=kvg[:].rearrange('p (t w) -> p t w', w=256)[:, :, 128:256]
                                .rearrange('p t (h q) -> p t h q', q=DH),
                            in1=scexp[:].rearrange('p (t h) -> p t h', h=H)
                                [:, :, :, None].to_broadcast([P, T, H, DH]))
                        aggp = ps_agg.tile([P, 132], F32, tag='aggp')
                        for i in range(T):
                            nc.tensor.matmul(aggp[:],
                                             lhsT=onehot[:, i * P:(i + 1) * P],
                                             rhs=work[:, i * 132:(i + 1) * 132],
                                             start=(i == 0), stop=(i == T - 1))
                        zr = edg.tile([P, H], F32, tag='zr')
                        nc.vector.tensor_scalar_add(out=zr[:], in0=aggp[:, 128:132],
                                                    scalar1=1e-16)
                        zrec = edg.tile([P, H], F32, tag='zrec')
                        nc.vector.reciprocal(out=zrec[:], in_=zr[:])
                        aggd = edg.tile([P, D], F32, tag='aggd')
                        nc.vector.tensor_mul(
                            out=aggd[:].rearrange('p (h q) -> p h q', q=DH),
                            in0=aggp[:, 0:128].rearrange('p (h q) -> p h q', q=DH),
                            in1=zrec[:, :, None].to_broadcast([P, H, DH]))
                        tps = ps_tps.tile([P, P], F32, tag='tps')
                        nc.tensor.transpose(out=tps[:], in_=aggd[:], identity=ident[:])
                        aggsb = edg.tile([P, P], F32, tag='aggsb')
                        nc.vector.tensor_copy(out=aggsb[:], in_=tps[:])
                        nc.sync.dma_start(out=aggT[t][:, b * P:(b + 1) * P],
                                          in_=aggsb[:])

                    # ======== out phase for this pass's destination type ====
                    # (t=0 rev->authors, t=1 papers->papers); layer-1 rev/out_a
                    # skipped entirely (h_a(2) is dead).
                    wo0 = wpool.tile([D, D], F32, tag='wo0')
                    nc.sync.dma_start(out=wo0[:], in_=Wout[l, t, :, :])
                    wo = wpool.tile([D, D], F32, tag='wo')
                    nc.scalar.activation(out=wo[:], in_=wo0[:],
                                         func=mybir.ActivationFunctionType.Identity)
                    bo = wpool.tile([D, 1], F32, tag='bo')
                    nc.sync.dma_start(out=bo[:], in_=Bout[l, t, :, :])
                    for (o, m) in chunksw:
                        ga = sb.tile([D, WCH], F32, tag='ga')
                        nc.sync.dma_start(out=ga[:, :m], in_=aggT[t][:, o:o + m])
                        gag = sb.tile([D, WCH], F32, tag='gag')
                        nc.scalar.activation(out=gag[:, :m], in_=ga[:, :m],
                                             func=mybir.ActivationFunctionType.Gelu)
                        ps = ps_mm.tile([P, WCH], F32, tag='mm')
                        nc.tensor.matmul(ps[:, :m], lhsT=wo[:], rhs=gag[:, :m],
                                         start=True, stop=True)
                        ob = sb.tile([D, WCH], F32, tag='ob')
                        nc.scalar.activation(out=ob[:, :m], in_=ps[:, :m],
                                             func=mybir.ActivationFunctionType.Identity,
                                             bias=bo[:])
                        hld = sb.tile([D, WCH], F32, tag='hld')
                        nc.sync.dma_start(out=hld[:, :m], in_=hT[t][:, o:o + m])
                        # h' = elu(s*o + (1-s)*h) = elu(h + s*(o-h))
                        dif = sb.tile([D, WCH], F32, tag='dif')
                        nc.vector.tensor_sub(out=dif[:, :m], in0=ob[:, :m],
                                             in1=hld[:, :m])
                        sd = sb.tile([D, WCH], F32, tag='sd')
                        nc.vector.tensor_scalar_mul(
                            out=sd[:, :m], in0=dif[:, :m],
                            scalar1=sig_t[:, l * 2 + t:l * 2 + t + 1])
                        hpre = sb.tile([D, WCH], F32, tag='hpre')
                        nc.vector.tensor_add(out=hpre[:, :m], in0=sd[:, :m],
                                             in1=hld[:, :m])
                        neg = sb.tile([D, WCH], F32, tag='neg')
                        nc.vector.tensor_scalar_min(out=neg[:, :m], in0=hpre[:, :m],
                                                    scalar1=0.0)
                        ex = sb.tile([D, WCH], F32, tag='ex')
                        nc.scalar.activation(out=ex[:, :m], in_=neg[:, :m],
                                             func=mybir.ActivationFunctionType.Exp)
                        rl = sb.tile([D, WCH], F32, tag='rl')
                        nc.vector.tensor_scalar_max(out=rl[:, :m], in0=hpre[:, :m],
                                                    scalar1=0.0)
                        er = sb.tile([D, WCH], F32, tag='er')
                        nc.vector.tensor_add(out=er[:, :m], in0=ex[:, :m],
                                             in1=rl[:, :m])
                        hnew = sb.tile([D, WCH], F32, tag='hnew')
                        nc.vector.tensor_scalar_add(out=hnew[:, :m], in0=er[:, :m],
                                                    scalar1=-1.0)
                        nc.sync.dma_start(out=hT[t][:, o:o + m], in_=hnew[:, :m])

            # ============ classifier ============
            cw0 = cpool.tile([D, OUT], F32)
            nc.sync.dma_start(out=cw0[:], in_=clsw[:, :])
            cw = cpool.tile([D, OUT], F32)
            nc.vector.tensor_copy(out=cw[:], in_=cw0[:])
            cb = cpool.tile([P, OUT], F32)
            nc.sync.dma_start(out=cb[:], in_=clsb[:, :])
            for (ow, mw) in chunksw:
                hc0 = sb.tile([D, WCH], F32, tag='hca0')
                nc.sync.dma_start(out=hc0[:, :mw], in_=hT[1][:, ow:ow + mw])
                hc = sb.tile([D, WCH], F32, tag='hca')
                nc.vector.tensor_copy(out=hc[:, :mw], in_=hc0[:, :mw])
                for oj in range(0, mw, P):
                    o = ow + oj
                    m = min(P, mw - oj)
                    ps = ps_mm.tile([P, OUT], F32, tag='mm')
                    nc.tensor.matmul(ps[:m, :], lhsT=hc[:, oj:oj + m], rhs=cw[:],
                                     start=True, stop=True)
                    lg = sb.tile([P, OUT], F32, tag='lg')
                    nc.vector.tensor_add(out=lg[:m, :], in0=ps[:m, :],
                                         in1=cb[:m, :])
                    nc.sync.dma_start(out=logits[o:o + m, :], in_=lg[:m, :])

    nc.finalize()
    return nc


# ---------------------------------------------------------------- entry point
_CACHE = {}


def kernel(**inputs):
    nn = np.asarray(inputs['x_author']).shape[0]
    nl = nn // C
    in_maps, TP, TR, nb, ncores = host_prep(inputs, nl)
    key = (nl, nb, TP, TR, ncores)
    if key not in _CACHE:
        _CACHE[key] = build_program(nl, nb, TP, TR, ncores)
    nc = _CACHE[key]
    res = run_bass_kernel_spmd(nc, in_maps, list(range(ncores)))
    outs = [res.results[c]['logits'] for c in range(ncores)]
    return np.concatenate(outs, 0)


if __name__ == '__main__':
    pass
